# revision 2
# baseline (speedup 1.0000x reference)
"""GQA causal attention (B=2, S=2048, D=1024, H=16, KVH=4) on 8 trn2 cores.

Core = (b, g): batch b, kv-head group g. Each core projects q (4 heads,
column-parallel), k/v (1 kv head), applies RoPE, runs causal attention, and
computes a row-parallel wo partial (bf16); the host sums 4 partials per batch.

v2 design (vs baseline): everything bf16 on the PE/DMA path (PSUM accum f32):
- bf16 matmuls run 1 cycle/row at ANY free size (no fp32r N>=256 penalty), so
  diagonal score blocks narrow to their exact valid windows.
- RoPE partner comes from a single 128x128 signed-permutation matmul on the
  projected q (1 extra N=512 matmul) instead of re-projecting against swapped
  weight columns (8 matmuls) — PE cost for rope drops 8x.
- k and v share one projection chain ([wk_deint | wv] stationary, k on
  partitions 0:63, v on 64:127); k is duplicated to the upper half by one
  SBUF-to-SBUF DMA for the odd heads' score matmuls.
- PV runs transposed: out[q,65] per (head, q-block) with probs as stationary —
  full 128-partition contraction (vs M=65 half-idle) halves PV column count;
  the 65th v column of ones accumulates the softmax denominators.
- normalization via per-partition tensor_scalar (no DRAM broadcast
  round-trip), then a PE transpose builds outT for the row-parallel wo.
- elementwise work is spread across DVE and GPSIMD so ACT only runs the exps.
PSUM: ring pool (scores/proj/wo, 2x2 banks) + transpose pool (1 bank) +
PV accumulators (8 slots, 3 banks) = 8 banks exactly.
"""

import numpy as np
import ml_dtypes

import concourse.bass as bass
import concourse.mybir as mybir
import concourse.tile as tile
from concourse.ap import AP
from concourse.bass_utils import run_bass_kernel_spmd

B, S, D = 2, 2048, 1024
H, KVH, HD = 16, 4, 64
GH = H // KVH          # 4 q heads per core
SC = 512               # q-chunk
NCH = S // SC          # 4 chunks
DT = D // 128          # 8 contraction tiles
F32 = mybir.dt.float32
F32R = mybir.dt.float32r
BF16 = mybir.dt.bfloat16
BF = ml_dtypes.bfloat16
Exp = mybir.ActivationFunctionType.Exp
MUL = None  # set in build


def build_nc(split=True, dbg=False):
    nc = bass.Bass("TRN2", target_bir_lowering=False, debug=False, num_devices=1)
    mul = mybir.AluOpType.mult
    add = mybir.AluOpType.add

    xt = nc.dram_tensor("xt", [D, S], BF16, kind="ExternalInput").ap()
    wq = nc.dram_tensor("wq", [D, GH * HD], BF16, kind="ExternalInput").ap()
    wkv = nc.dram_tensor("wkv", [D, 2 * HD], BF16, kind="ExternalInput").ap()
    wo = nc.dram_tensor("wo", [GH * HD, D], BF16, kind="ExternalInput").ap()
    cos4 = nc.dram_tensor("cos4", [128, S], BF16, kind="ExternalInput").ap()
    sin4 = nc.dram_tensor("sin4", [128, S], BF16, kind="ExternalInput").ap()
    m01 = nc.dram_tensor("m01", [128, 128], BF16, kind="ExternalInput").ap()
    pmat = nc.dram_tensor("pmat", [128, 128], BF16, kind="ExternalInput").ap()
    ident = nc.dram_tensor("ident", [128, 128], BF16, kind="ExternalInput").ap()
    out = nc.dram_tensor("out", [S, D], BF16, kind="ExternalOutput").ap()
    if dbg:
        dqr = nc.dram_tensor("dqr", [128, SC], BF16, kind="ExternalOutput").ap()
        dkr = nc.dram_tensor("dkr", [128, SC], BF16, kind="ExternalOutput").ap()
        dv0 = nc.dram_tensor("dv0", [128, HD + 1], BF16, kind="ExternalOutput").ap()
        dpt = nc.dram_tensor("dpt", [128, 2 * SC], BF16, kind="ExternalOutput").ap()
        don = nc.dram_tensor("don", [128, 128], BF16, kind="ExternalOutput").ap()
        doT = nc.dram_tensor("doT", [128, SC], BF16, kind="ExternalOutput").ap()

    with tile.TileContext(nc) as tc:
        from contextlib import ExitStack

        with ExitStack() as ctx:
            singles = ctx.enter_context(tc.tile_pool(name="singles", bufs=1))
            persist = ctx.enter_context(tc.tile_pool(name="persist", bufs=1))
            qd_pool = ctx.enter_context(tc.tile_pool(name="qd", bufs=2))
            tc_pool = ctx.enter_context(tc.tile_pool(name="tcs", bufs=2))
            qrot_pool = ctx.enter_context(tc.tile_pool(name="qrot", bufs=2))
            vt_pool = ctx.enter_context(tc.tile_pool(name="vt", bufs=2))
            pt_pool = ctx.enter_context(tc.tile_pool(name="pt", bufs=8))
            rec_pool = ctx.enter_context(tc.tile_pool(name="rec", bufs=2))
            on_pool = ctx.enter_context(tc.tile_pool(name="on", bufs=8))
            outT_pool = ctx.enter_context(tc.tile_pool(name="outT", bufs=2))
            stage_pool = ctx.enter_context(tc.tile_pool(name="stage", bufs=2))
            ring = ctx.enter_context(tc.tile_pool(name="ring", bufs=3, space="PSUM"))
            pv = ctx.enter_context(tc.tile_pool(name="pv", bufs=1, space="PSUM"))

            # ---- batched loads: the 128-partition-row tiles of each DRAM
            # tensor are packed side by side in one SBUF tile and shipped with
            # a single 3-dim DMA (HWDGE issue slots, not bytes, dominate the
            # startup). Chunk-0's x/cos/sin columns come first so the first
            # projection chain starts within a few us. ----
            def load_packed(name, dram, rows, width, eng, cols=None):
                nblk = rows // 128
                t = singles.tile([128, nblk * width], BF16, tag=name, name=name)
                c0, c1 = (0, width) if cols is None else cols
                w = c1 - c0
                in_ap = AP(dram.tensor, c0, [[width, 128], [128 * width, nblk], [1, w]])
                rs = t[:, :].ap[0][0]
                out_ap = AP(t[:, :].tensor, t[0:1, 0:1].offset + c0,
                            [[rs, 128], [width, nblk], [1, w]])
                eng.dma_start(out=out_ap, in_=in_ap)
                return t

            wq_all = load_packed("wqa", wq, D, GH * HD, nc.sync)
            xt_all = load_packed("xta", xt, D, S, nc.scalar, cols=(0, SC))
            sincos_sb = singles.tile([128, 2 * S], BF16, tag="sincos")
            sin_sb = sincos_sb[:, 0:S]
            cos_sb = sincos_sb[:, S : 2 * S]
            nc.sync.dma_start(out=sin_sb[0:128, 0:SC], in_=sin4[:, 0:SC])
            nc.sync.dma_start(out=cos_sb[0:128, 0:SC], in_=cos4[:, 0:SC])
            pmat_sb = singles.tile([128, 128], BF16, tag="pmat")
            nc.sync.dma_start(out=pmat_sb, in_=pmat)
            wkv_all = load_packed("wkva", wkv, D, 2 * HD, nc.sync)
            ident_sb = singles.tile([128, 128], BF16, tag="ident")
            nc.sync.dma_start(out=ident_sb, in_=ident)
            m01_sb = singles.tile([128, 128], BF16, tag="m01")
            nc.sync.dma_start(out=m01_sb, in_=m01)
            # bulk loads are deprioritized so the scheduler keeps them off
            # the startup critical path (DMA_ENGINES is a serial resource);
            # the x remainder ships in per-chunk pieces to bound head-of-line
            # blocking
            with tc.high_priority(offset=-100000):
                nc.sync.dma_start(out=cos_sb[0:128, SC:S], in_=cos4[:, SC:S])
                nc.sync.dma_start(out=sin_sb[0:128, SC:S], in_=sin4[:, SC:S])
                rs = xt_all[:, :].ap[0][0]
                for cpiece in range(1, NCH):
                    c0 = SC * cpiece
                    in2 = AP(xt.tensor, c0, [[S, 128], [128 * S, DT], [1, SC]])
                    out2 = AP(xt_all[:, :].tensor, xt_all[0:1, 0:1].offset + c0,
                              [[rs, 128], [S, DT], [1, SC]])
                    nc.scalar.dma_start(out=out2, in_=in2)
                wo_all = load_packed("woa", wo, GH * HD, D, nc.sync)

            junk_sb = singles.tile([128, 64], BF16, tag="junk")
            nc.vector.memset(junk_sb, 0.0)
            for w in range(40):
                junk_ps = pv.tile([128, 642], F32, tag="pv", name="junk_ps")
                nc.tensor.matmul(
                    junk_ps[0:64, 0:64], junk_sb, junk_sb, start=True, stop=True
                )

            wq_sb = [wq_all[:, GH * HD * dt : GH * HD * (dt + 1)] for dt in range(DT)]
            xt_sb = [xt_all[:, S * dt : S * (dt + 1)] for dt in range(DT)]
            wkv_sb = [wkv_all[:, 2 * HD * dt : 2 * HD * (dt + 1)] for dt in range(DT)]
            wo_sb = [wo_all[:, D * r : D * (r + 1)] for r in range(2)]

            # ---- persistent activations ----
            krot = [
                persist.tile([128, SC], BF16, tag=f"krot{c}", name=f"krot{c}")
                for c in range(NCH)
            ]
            v_sb = [
                persist.tile([128, HD + 1], BF16, tag=f"v{kb}", name=f"v{kb}")
                for kb in range(S // 128)
            ]
            for kb in range(S // 128):
                nc.gpsimd.memset(v_sb[kb][:, HD : HD + 1], 1.0)

            def rope(ps, rows, cs, dst):
                """dst = (ps*cos) + P @ (ps*sin), exploiting that cos/sin are
                partner-symmetric across the 32-row swap. All bf16 out."""
                d_sin = qd_pool.tile([rows, SC], BF16, tag="qd", name="d_sin")
                nc.vector.tensor_tensor(d_sin, ps[0:rows, :], sin_sb[0:rows, cs], mul)
                d_cos = tc_pool.tile([rows, SC], BF16, tag="tc", name="d_cos")
                nc.vector.tensor_tensor(d_cos, ps[0:rows, :], cos_sb[0:rows, cs], mul)
                s_ps = ring.tile([rows, SC], F32, tag="mm", name="s_ps")
                nc.tensor.matmul(
                    s_ps, pmat_sb[0:rows, 0:rows], d_sin, start=True, stop=True
                )
                nc.vector.tensor_tensor(dst, s_ps, d_cos, add)

            stage_hold = {}

            def emit_wo_qb(c, outT_c, qb):
                """One 128-row output block: 2 accumulating matmuls into a pv-pool
                psum tile, evacuated to a shared bf16 stage (DVE/ACT alternating);
                one batched DMA per chunk ships all 4 blocks."""
                w_ps = ring.tile([128, D], F32, tag="mm", name="w_ps")
                for n in range(2):
                    for r in range(2):
                        nc.tensor.matmul(
                            w_ps[:, 512 * n : 512 * (n + 1)],
                            outT_c[r][:, 128 * qb : 128 * (qb + 1)],
                            wo_sb[r][:, 512 * n : 512 * (n + 1)],
                            start=(r == 0),
                            stop=(r == 1),
                        )
                if qb == 0:
                    stage_hold["t"] = stage_pool.tile(
                        [128, 4 * D], BF16, tag="stage", name="stage"
                    )
                stage = stage_hold["t"]
                if c + 1 < NCH:
                    # low-priority on DVE: the scheduler slots these 1.2us
                    # copies into DVE idle windows off the ACT exp stream
                    with tc.high_priority(offset=-2000):
                        nc.vector.tensor_copy(stage[:, D * qb : D * (qb + 1)], w_ps)
                else:
                    # critical tail: split halves across DVE and ACT
                    h = D // 2
                    nc.vector.tensor_copy(
                        stage[:, D * qb : D * qb + h], w_ps[:, 0:h]
                    )
                    nc.scalar.copy(stage[:, D * qb + h : D * (qb + 1)], w_ps[:, h:D])
                r0 = SC * c + 128 * qb
                if c + 1 < NCH:
                    eng = nc.sync if qb % 2 == 0 else nc.gpsimd
                else:
                    eng = nc.sync if qb % 2 == 0 else nc.scalar
                eng.dma_start(out=out[r0 : r0 + 128, :], in_=stage[:, D * qb : D * (qb + 1)])

            # ---- deferred-emission machinery: projection chains for chunk c+1
            # and wo blocks for chunk c-1 are emitted as background thunks
            # interleaved into chunk c's kb loop, so the PE fills exp-latency
            # bubbles with projection/wo work and ACT never starves ----
            qrot_by_c = {}

            def proj_thunks(c):
                cs = slice(SC * c, SC * (c + 1))
                qrot_by_c[c] = {}

                def q_chain(p):
                    def f():
                        t = qrot_pool.tile(
                            [128, SC], BF16, tag=f"qr{p}", name=f"qr{p}"
                        )
                        qrot_by_c[c][p] = t
                        q_ps = ring.tile([128, SC], F32, tag="mm", name="q_ps")
                        for dt in range(DT):
                            nc.tensor.matmul(
                                q_ps,
                                wq_sb[dt][:, 128 * p : 128 * (p + 1)],
                                xt_sb[dt][:, cs],
                                start=(dt == 0),
                                stop=(dt == DT - 1),
                            )
                        rope(q_ps, 128, cs, t)
                    return f

                def kv_chain():
                    kv_ps = ring.tile([128, SC], F32, tag="mm", name="kv_ps")
                    for dt in range(DT):
                        nc.tensor.matmul(
                            kv_ps,
                            wkv_sb[dt],
                            xt_sb[dt][:, cs],
                            start=(dt == 0),
                            stop=(dt == DT - 1),
                        )
                    rope(kv_ps, 64, cs, krot[c][0:64, :])
                    # duplicate k to the upper partition half for odd heads
                    nc.sync.dma_start(out=krot[c][64:128, :], in_=krot[c][0:64, :])
                    # v: evacuate for later transposes
                    vT = vt_pool.tile([64, SC], BF16, tag="vT", name=f"vT{c}")
                    nc.vector.tensor_copy(vT, kv_ps[64:128, :])
                    kv_hold[c] = vT

                def v_trans(j):
                    def f():
                        vt_ps = ring.tile([128, HD], BF16, tag="mm", name="vt_ps")
                        nc.tensor.transpose(
                            vt_ps,
                            kv_hold[c][:, 128 * j : 128 * (j + 1)],
                            ident_sb[0:64, 0:64],
                        )
                        nc.vector.tensor_copy(v_sb[4 * c + j][:, 0:HD], vt_ps)
                    return f

                return [q_chain(0), q_chain(1), kv_chain] + [
                    v_trans(j) for j in range(4)
                ]

            kv_hold = {}
            fin_hold = {}
            pending_wo = None
            # prologue: chunk 0 projections run up front
            for th in proj_thunks(0):
                th()

            for c in range(NCH):
                # background work to interleave into this chunk's attention:
                # wo blocks of chunk c-1 and projections of chunk c+1
                wo_th = []
                if pending_wo is not None:
                    pw = pending_wo
                    wo_th = [
                        (lambda qb: lambda: emit_wo_qb(pw[0], pw[1], qb))(qb)
                        for qb in range(4)
                    ]
                pj_th = proj_thunks(c + 1) if c + 1 < NCH else []
                bg = [x for pair in zip(
                    wo_th + [None] * len(pj_th), pj_th + [None] * len(wo_th)
                ) for x in pair if x is not None]

                bg = fin_hold.pop("t", []) + bg
                qrot = qrot_by_c[c]
                # ---- attention ----
                outT_c = [
                    outT_pool.tile([128, SC], BF16, tag=f"oT{r}", name=f"oT{r}")
                    for r in range(2)
                ]
                if dbg and c == 0:
                    nc.sync.dma_start(out=dqr, in_=qrot[0])
                    nc.sync.dma_start(out=dkr, in_=krot[0])
                    nc.sync.dma_start(out=dv0, in_=v_sb[0])
                nkb = 4 * c + 4
                # one flattened, software-pipelined stream over (hp, kb):
                # PV(kb) trails scores by 2 kb groups so the PE never waits on
                # exp; retirement normalization is DVE-only and inline; all 8
                # transposes retire at chunk end (deps long satisfied by then)
                hp_state = {}
                o_ns = {}
                retire_q = []

                def emit_pv(hp, kb, j, pt2, hp_state=hp_state, retire_q=retire_q):
                    if kb == 0:
                        # qb3 slot starts at col 512 so no 65-col accumulation
                        # region crosses a PSUM bank boundary (start=True
                        # zeroing does not reach across banks)
                        pv_hp = pv.tile([128, 642], F32, tag="pv", name="pv_hp")
                        base = {0: 0, 1: 130, 2: 260, 3: 512}
                        hp_state[hp] = {
                            qb: pv_hp[:, base[qb] : base[qb] + 130]
                            for qb in range(4)
                        }
                    slots = hp_state[hp]
                    for qb in range(max(j, 0), 4):
                        for hi in range(2):
                            # PSUM start=True zeroes a whole 2KB zero-region
                            # (bank), so only ONE group may open per bank:
                            # start on the first matmul into each bank
                            # (qb0 -> bank0, qb3 -> bank1), stop on the last
                            # (qb2/qb3 diag, hi=1). Intermediate slots get
                            # first-touch-overwrite via per-byte pending-zero.
                            nc.tensor.matmul(
                                slots[qb][:, 65 * hi : 65 * (hi + 1)],
                                pt2[:, SC * hi + 128 * qb : SC * hi + 128 * (qb + 1)],
                                v_sb[kb][:, 0 : HD + 1],
                                start=(kb == 0 and hi == 0 and qb in (0, 3)),
                                stop=(j == qb and hi == 1 and qb in (2, 3)),
                                skip_group_check=True,
                            )
                        if j == qb:
                            retire_q.append((hp, qb))

                def run_retires(retire_q=retire_q, hp_state=hp_state, o_ns=o_ns, c=c):
                    while retire_q:
                        hp_, qb = retire_q.pop(0)
                        slot = hp_state[hp_][qb]
                        rs = slot[:, :].ap[0][0]
                        den = AP(
                            slot[:, :].tensor,
                            slot[0:1, 0:1].offset + HD,
                            [[rs, 128], [65, 2]],
                        )
                        rec = rec_pool.tile([128, 2], F32, tag="rec", name="rec")
                        with nc.allow_low_precision(reason="softmax denom"):
                            nc.vector.reciprocal(rec, den)
                        o_n = on_pool.tile([128, 128], BF16, tag="on", name="o_n")
                        for hi in range(2):
                            nc.vector.tensor_scalar(
                                o_n[:, 64 * hi : 64 * (hi + 1)],
                                slot[:, 65 * hi : 65 * hi + HD],
                                rec[:, hi : hi + 1],
                                None,
                                mul,
                            )
                        o_ns[(hp_, qb)] = o_n
                        if dbg and c == 0 and hp_ == 0 and qb == 0:
                            nc.sync.dma_start(out=don, in_=o_n)

                pends = []
                for hp in range(2):
                    for kb in range(nkb):
                        j = kb - 4 * c
                        col0 = max(0, 128 * j)
                        s2 = ring.tile([128, 2 * SC], F32, tag="mm", name="s2")
                        pt2 = pt_pool.tile([128, 2 * SC], BF16, tag="pt", name="pt2")
                        for hi in range(2):
                            nc.tensor.matmul(
                                s2[:, SC * hi + col0 : SC * (hi + 1)],
                                krot[kb // 4][
                                    64 * hi : 64 * (hi + 1),
                                    128 * (kb % 4) : 128 * (kb % 4 + 1),
                                ],
                                qrot[hp][64 * hi : 64 * (hi + 1), col0:SC],
                                start=True,
                                stop=True,
                            )
                        if col0 == 0:
                            nc.scalar.activation(pt2, s2, Exp, scale=0.125)
                        else:
                            s2v = s2.rearrange("a (t q) -> a t q", t=2)[:, :, col0:]
                            pt2v = pt2.rearrange("a (t q) -> a t q", t=2)[:, :, col0:]
                            nc.scalar.activation(pt2v, s2v, Exp, scale=0.125)
                        if j >= 0:
                            for hi in range(2):
                                w = slice(SC * hi + col0, SC * hi + col0 + 128)
                                nc.vector.tensor_tensor(
                                    pt2[:, w], pt2[:, w], m01_sb, mul
                                )
                        if dbg and c == 0 and hp == 0 and kb == 0:
                            nc.sync.dma_start(out=dpt, in_=pt2)
                        pends.append((hp, kb, j, pt2))
                        cap = 6 if kb < 8 else 2
                        if len(pends) > cap:
                            emit_pv(*pends.pop(0))
                        if kb == 1 and retire_q:
                            run_retires()
                        # slot one background thunk (next-chunk proj / prev-chunk
                        # wo) between kb groups to fill exp-latency bubbles
                        # (small chunks have few kb slots, so pop every kb there)
                        if bg and kb % 2 == 1:
                            bg.pop(0)()
                for p_ in pends:
                    emit_pv(*p_)
                if c + 1 < NCH:
                    def tr_bundle(oc, onsd, hp, cc):
                        def f():
                            for qb in range(4):
                                tr_ps = ring.tile(
                                    [128, 128], BF16, tag="mm", name="tr_ps"
                                )
                                nc.tensor.transpose(tr_ps, onsd[(hp, qb)], ident_sb)
                                nc.vector.tensor_copy(
                                    oc[hp][:, 128 * qb : 128 * (qb + 1)], tr_ps
                                )
                            if dbg and cc == 0 and hp == 0:
                                nc.sync.dma_start(out=doT, in_=oc[0])
                        return f

                    fin_hold["t"] = [run_retires] + [
                        tr_bundle(outT_c, o_ns, hp, c) for hp in range(2)
                    ]
                else:
                    # last chunk: drain leftover background first, then
                    # retire all, then per-qb transpose -> wo -> ship
                    while bg:
                        bg.pop(0)()
                    run_retires()
                    for qb in range(4):
                        for hp in range(2):
                            tr_ps = ring.tile([128, 128], BF16, tag="mm", name="tr_ps")
                            nc.tensor.transpose(tr_ps, o_ns[(hp, qb)], ident_sb)
                            nc.vector.tensor_copy(
                                outT_c[hp][:, 128 * qb : 128 * (qb + 1)], tr_ps
                            )
                        emit_wo_qb(c, outT_c, qb)
                # drain leftover background thunks (small chunks have few slots)
                for th in bg:
                    th()
                pending_wo = (c, outT_c)

    if split:
        split_excess_waits(nc)
    return nc


def split_excess_waits(nc, max_waits=1):
    """This container's walrus codegen supports one semaphore wait per
    instruction. Hoist excess waits onto NOPs injected just before, on the
    same engine (engine program order preserves the semantics)."""
    n_split = 0
    for fn in nc.m.functions:
        for bb in fn.blocks:
            insts = bb.instructions
            new = []
            for inst in insts:
                si = inst.sync_info
                waits = list(si.on_wait) if si is not None and si.on_wait else []
                if len(waits) > max_waits:
                    n_split += 1
                    extra, keep = waits[:-max_waits], waits[-max_waits:]
                    for k in range(0, len(extra), max_waits):
                        nop = mybir.InstNoOp(
                            name=nc.get_next_instruction_name(), ins=[], outs=[]
                        )
                        nop.engine = inst.engine
                        nop.sync_info = mybir.SyncInfo(
                            on_wait=extra[k : k + max_waits], on_update=[]
                        )
                        new.append(nop)
                    inst.sync_info = mybir.SyncInfo(
                        on_wait=keep,
                        on_update=list(si.on_update) if si.on_update else [],
                    )
                new.append(inst)
            bb.instructions = new
    return n_split


def _host_shards(x, wq, wk, wv, wo, freqs_cos, freqs_sin, mask):
    deint = np.concatenate([np.arange(0, HD, 2), np.arange(1, HD, 2)])

    cosb = np.tile(freqs_cos.T, (4, 1)).astype(BF)         # [128, S] rows r: cos_{r%32}
    sinb = np.tile(freqs_sin.T, (4, 1)).astype(BF)
    # signed swap permutation: (P @ t)[r] = sign(r) * t[r ^ 32],
    # sign = -1 for r%64 < 32 (the "or = tr*c - ti*s" half), +1 otherwise
    sign = np.where((np.arange(128) % 64) < 32, -1.0, 1.0).astype(np.float32)
    partner = np.arange(128) ^ 32
    P = np.zeros((128, 128), dtype=np.float32)
    P[partner, np.arange(128)] = sign
    m01 = (np.arange(128)[None, :] >= np.arange(128)[:, None]).astype(np.float32)
    ident = np.eye(128, dtype=np.float32)

    xts = [np.ascontiguousarray(x[b].T).astype(BF) for b in range(B)]

    in_maps = []
    for core in range(8):
        b, g = divmod(core, KVH)
        wq_g = wq[:, g * GH * HD : (g + 1) * GH * HD]
        wq_d = np.concatenate(
            [wq_g[:, h * HD + deint] for h in range(GH)], axis=1
        ).astype(BF)
        wk_g = wk[:, g * HD : (g + 1) * HD][:, deint]
        wv_g = wv[:, g * HD : (g + 1) * HD]
        wkv_g = np.concatenate([wk_g, wv_g], axis=1).astype(BF)
        wo_g = wo[g * GH * HD : (g + 1) * GH * HD, :].astype(BF)
        in_maps.append(
            {
                "xt": xts[b],
                "wq": np.ascontiguousarray(wq_d),
                "wkv": np.ascontiguousarray(wkv_g),
                "wo": np.ascontiguousarray(wo_g),
                "cos4": np.ascontiguousarray(cosb),
                "sin4": np.ascontiguousarray(sinb),
                "m01": m01.astype(BF),
                "pmat": P.astype(BF),
                "ident": ident.astype(BF),
            }
        )
    return in_maps


_NC_CACHE = None


def get_nc():
    global _NC_CACHE
    if _NC_CACHE is None:
        _NC_CACHE = build_nc()
    return _NC_CACHE


def kernel(x, wq, wk, wv, wo, freqs_cos, freqs_sin, mask):
    in_maps = _host_shards(
        np.asarray(x),
        np.asarray(wq),
        np.asarray(wk),
        np.asarray(wv),
        np.asarray(wo),
        np.asarray(freqs_cos),
        np.asarray(freqs_sin),
        np.asarray(mask),
    )
    nc = get_nc()
    res = run_bass_kernel_spmd(nc, in_maps, core_ids=list(range(8)))
    parts = [np.asarray(res.results[i]["out"], dtype=np.float32) for i in range(8)]
    out = np.stack(
        [
            parts[0] + parts[1] + parts[2] + parts[3],
            parts[4] + parts[5] + parts[6] + parts[7],
        ]
    ).astype(np.float32)
    return out


# revision 3
# speedup vs baseline: 1.0181x; 1.0181x over previous
"""GQA causal attention (B=2, S=2048, D=1024, H=16, KVH=4) on 8 trn2 cores.

Core = (b, g): batch b, kv-head group g. Each core projects q (4 heads,
column-parallel), k/v (1 kv head), applies RoPE, runs causal attention, and
computes a row-parallel wo partial (bf16); the host sums 4 partials per batch.

v2 design (vs baseline): everything bf16 on the PE/DMA path (PSUM accum f32):
- bf16 matmuls run 1 cycle/row at ANY free size (no fp32r N>=256 penalty), so
  diagonal score blocks narrow to their exact valid windows.
- RoPE partner comes from a single 128x128 signed-permutation matmul on the
  projected q (1 extra N=512 matmul) instead of re-projecting against swapped
  weight columns (8 matmuls) — PE cost for rope drops 8x.
- k and v share one projection chain ([wk_deint | wv] stationary, k on
  partitions 0:63, v on 64:127); k is duplicated to the upper half by one
  SBUF-to-SBUF DMA for the odd heads' score matmuls.
- PV runs transposed: out[q,65] per (head, q-block) with probs as stationary —
  full 128-partition contraction (vs M=65 half-idle) halves PV column count;
  the 65th v column of ones accumulates the softmax denominators.
- normalization via per-partition tensor_scalar (no DRAM broadcast
  round-trip), then a PE transpose builds outT for the row-parallel wo.
- elementwise work is spread across DVE and GPSIMD so ACT only runs the exps.
PSUM: ring pool (scores/proj/wo, 2x2 banks) + transpose pool (1 bank) +
PV accumulators (8 slots, 3 banks) = 8 banks exactly.
"""

import numpy as np
import ml_dtypes

import concourse.bass as bass
import concourse.mybir as mybir
import concourse.tile as tile
from concourse.ap import AP
from concourse.bass_utils import run_bass_kernel_spmd

B, S, D = 2, 2048, 1024
H, KVH, HD = 16, 4, 64
GH = H // KVH          # 4 q heads per core
SC = 512               # q-chunk
NCH = S // SC          # 4 chunks
DT = D // 128          # 8 contraction tiles
F32 = mybir.dt.float32
F32R = mybir.dt.float32r
BF16 = mybir.dt.bfloat16
BF = ml_dtypes.bfloat16
Exp = mybir.ActivationFunctionType.Exp
MUL = None  # set in build


def build_nc(split=True, dbg=False):
    nc = bass.Bass("TRN2", target_bir_lowering=False, debug=False, num_devices=1)
    mul = mybir.AluOpType.mult
    add = mybir.AluOpType.add

    xt = nc.dram_tensor("xt", [D, S], BF16, kind="ExternalInput").ap()
    wq = nc.dram_tensor("wq", [D, GH * HD], BF16, kind="ExternalInput").ap()
    wkv = nc.dram_tensor("wkv", [D, 2 * HD], BF16, kind="ExternalInput").ap()
    wo = nc.dram_tensor("wo", [GH * HD, D], BF16, kind="ExternalInput").ap()
    cos4 = nc.dram_tensor("cos4", [128, S], BF16, kind="ExternalInput").ap()
    sin4 = nc.dram_tensor("sin4", [128, S], BF16, kind="ExternalInput").ap()
    m01 = nc.dram_tensor("m01", [128, 128], BF16, kind="ExternalInput").ap()
    pmat = nc.dram_tensor("pmat", [128, 128], BF16, kind="ExternalInput").ap()
    ident = nc.dram_tensor("ident", [128, 128], BF16, kind="ExternalInput").ap()
    out = nc.dram_tensor("out", [S, D], BF16, kind="ExternalOutput").ap()
    if dbg:
        dqr = nc.dram_tensor("dqr", [128, SC], BF16, kind="ExternalOutput").ap()
        dkr = nc.dram_tensor("dkr", [128, SC], BF16, kind="ExternalOutput").ap()
        dv0 = nc.dram_tensor("dv0", [128, HD + 1], BF16, kind="ExternalOutput").ap()
        dpt = nc.dram_tensor("dpt", [128, 2 * SC], BF16, kind="ExternalOutput").ap()
        don = nc.dram_tensor("don", [128, 128], BF16, kind="ExternalOutput").ap()
        doT = nc.dram_tensor("doT", [128, SC], BF16, kind="ExternalOutput").ap()

    with tile.TileContext(nc) as tc:
        from contextlib import ExitStack

        with ExitStack() as ctx:
            singles = ctx.enter_context(tc.tile_pool(name="singles", bufs=1))
            persist = ctx.enter_context(tc.tile_pool(name="persist", bufs=1))
            qd_pool = ctx.enter_context(tc.tile_pool(name="qd", bufs=2))
            tc_pool = ctx.enter_context(tc.tile_pool(name="tcs", bufs=2))
            qrot_pool = ctx.enter_context(tc.tile_pool(name="qrot", bufs=2))
            vt_pool = ctx.enter_context(tc.tile_pool(name="vt", bufs=2))
            pt_pool = ctx.enter_context(tc.tile_pool(name="pt", bufs=8))
            rec_pool = ctx.enter_context(tc.tile_pool(name="rec", bufs=2))
            on_pool = ctx.enter_context(tc.tile_pool(name="on", bufs=8))
            outT_pool = ctx.enter_context(tc.tile_pool(name="outT", bufs=2))
            stage_pool = ctx.enter_context(tc.tile_pool(name="stage", bufs=2))
            ring = ctx.enter_context(tc.tile_pool(name="ring", bufs=3, space="PSUM"))
            pv = ctx.enter_context(tc.tile_pool(name="pv", bufs=1, space="PSUM"))

            # ---- batched loads: the 128-partition-row tiles of each DRAM
            # tensor are packed side by side in one SBUF tile and shipped with
            # a single 3-dim DMA (HWDGE issue slots, not bytes, dominate the
            # startup). Chunk-0's x/cos/sin columns come first so the first
            # projection chain starts within a few us. ----
            def load_packed(name, dram, rows, width, eng, cols=None):
                nblk = rows // 128
                t = singles.tile([128, nblk * width], BF16, tag=name, name=name)
                c0, c1 = (0, width) if cols is None else cols
                w = c1 - c0
                in_ap = AP(dram.tensor, c0, [[width, 128], [128 * width, nblk], [1, w]])
                rs = t[:, :].ap[0][0]
                out_ap = AP(t[:, :].tensor, t[0:1, 0:1].offset + c0,
                            [[rs, 128], [width, nblk], [1, w]])
                eng.dma_start(out=out_ap, in_=in_ap)
                return t

            wq_all = load_packed("wqa", wq, D, GH * HD, nc.sync)
            xt_all = load_packed("xta", xt, D, S, nc.scalar, cols=(0, SC))
            sincos_sb = singles.tile([128, 2 * S], BF16, tag="sincos")
            sin_sb = sincos_sb[:, 0:S]
            cos_sb = sincos_sb[:, S : 2 * S]
            nc.sync.dma_start(out=sin_sb[0:128, 0:SC], in_=sin4[:, 0:SC])
            nc.sync.dma_start(out=cos_sb[0:128, 0:SC], in_=cos4[:, 0:SC])
            pmat_sb = singles.tile([128, 128], BF16, tag="pmat")
            nc.sync.dma_start(out=pmat_sb, in_=pmat)
            wkv_all = load_packed("wkva", wkv, D, 2 * HD, nc.sync)
            ident_sb = singles.tile([128, 128], BF16, tag="ident")
            nc.sync.dma_start(out=ident_sb, in_=ident)
            m01_sb = singles.tile([128, 128], BF16, tag="m01")
            nc.sync.dma_start(out=m01_sb, in_=m01)
            # bulk loads are deprioritized so the scheduler keeps them off
            # the startup critical path (DMA_ENGINES is a serial resource);
            # the x remainder ships in per-chunk pieces to bound head-of-line
            # blocking
            with tc.high_priority(offset=-100000):
                nc.sync.dma_start(out=cos_sb[0:128, SC:S], in_=cos4[:, SC:S])
                nc.sync.dma_start(out=sin_sb[0:128, SC:S], in_=sin4[:, SC:S])
                rs = xt_all[:, :].ap[0][0]
                for cpiece in range(1, NCH):
                    c0 = SC * cpiece
                    in2 = AP(xt.tensor, c0, [[S, 128], [128 * S, DT], [1, SC]])
                    out2 = AP(xt_all[:, :].tensor, xt_all[0:1, 0:1].offset + c0,
                              [[rs, 128], [S, DT], [1, SC]])
                    nc.scalar.dma_start(out=out2, in_=in2)
                wo_all = load_packed("woa", wo, GH * HD, D, nc.sync)

            junk_sb = singles.tile([128, 64], BF16, tag="junk")
            nc.vector.memset(junk_sb, 0.0)
            for w in range(40):
                junk_ps = pv.tile([128, 642], F32, tag="pv", name="junk_ps")
                nc.tensor.matmul(
                    junk_ps[0:64, 0:64], junk_sb, junk_sb, start=True, stop=True
                )

            wq_sb = [wq_all[:, GH * HD * dt : GH * HD * (dt + 1)] for dt in range(DT)]
            xt_sb = [xt_all[:, S * dt : S * (dt + 1)] for dt in range(DT)]
            wkv_sb = [wkv_all[:, 2 * HD * dt : 2 * HD * (dt + 1)] for dt in range(DT)]
            wo_sb = [wo_all[:, D * r : D * (r + 1)] for r in range(2)]

            # ---- persistent activations ----
            krot = [
                persist.tile([128, SC], BF16, tag=f"krot{c}", name=f"krot{c}")
                for c in range(NCH)
            ]
            v_sb = [
                persist.tile([128, HD + 1], BF16, tag=f"v{kb}", name=f"v{kb}")
                for kb in range(S // 128)
            ]
            for kb in range(S // 128):
                nc.gpsimd.memset(v_sb[kb][:, HD : HD + 1], 1.0)

            def rope(ps, rows, cs, dst):
                """dst = (ps*cos) + P @ (ps*sin), exploiting that cos/sin are
                partner-symmetric across the 32-row swap. All bf16 out."""
                d_sin = qd_pool.tile([rows, SC], BF16, tag="qd", name="d_sin")
                nc.vector.tensor_tensor(d_sin, ps[0:rows, :], sin_sb[0:rows, cs], mul)
                d_cos = tc_pool.tile([rows, SC], BF16, tag="tc", name="d_cos")
                nc.vector.tensor_tensor(d_cos, ps[0:rows, :], cos_sb[0:rows, cs], mul)
                s_ps = ring.tile([rows, SC], F32, tag="mm", name="s_ps")
                nc.tensor.matmul(
                    s_ps, pmat_sb[0:rows, 0:rows], d_sin, start=True, stop=True
                )
                nc.vector.tensor_tensor(dst, s_ps, d_cos, add)

            stage_hold = {}

            def emit_wo_qb(c, outT_c, qb):
                """One 128-row output block: 2 accumulating matmuls into a pv-pool
                psum tile, evacuated to a shared bf16 stage (DVE/ACT alternating);
                one batched DMA per chunk ships all 4 blocks."""
                w_ps = ring.tile([128, D], F32, tag="mm", name="w_ps")
                for n in range(2):
                    for r in range(2):
                        nc.tensor.matmul(
                            w_ps[:, 512 * n : 512 * (n + 1)],
                            outT_c[r][:, 128 * qb : 128 * (qb + 1)],
                            wo_sb[r][:, 512 * n : 512 * (n + 1)],
                            start=(r == 0),
                            stop=(r == 1),
                        )
                if qb == 0:
                    stage_hold["t"] = stage_pool.tile(
                        [128, 4 * D], BF16, tag="stage", name="stage"
                    )
                stage = stage_hold["t"]
                if c + 1 < NCH:
                    # low-priority on DVE: the scheduler slots these 1.2us
                    # copies into DVE idle windows off the ACT exp stream
                    with tc.high_priority(offset=-2000):
                        nc.vector.tensor_copy(stage[:, D * qb : D * (qb + 1)], w_ps)
                else:
                    # critical tail: split halves across DVE and ACT
                    h = D // 2
                    nc.vector.tensor_copy(
                        stage[:, D * qb : D * qb + h], w_ps[:, 0:h]
                    )
                    nc.scalar.copy(stage[:, D * qb + h : D * (qb + 1)], w_ps[:, h:D])
                r0 = SC * c + 128 * qb
                if c + 1 < NCH:
                    eng = nc.sync if qb % 2 == 0 else nc.gpsimd
                else:
                    eng = nc.sync if qb % 2 == 0 else nc.scalar
                eng.dma_start(out=out[r0 : r0 + 128, :], in_=stage[:, D * qb : D * (qb + 1)])

            # ---- deferred-emission machinery: projection chains for chunk c+1
            # and wo blocks for chunk c-1 are emitted as background thunks
            # interleaved into chunk c's kb loop, so the PE fills exp-latency
            # bubbles with projection/wo work and ACT never starves ----
            qrot_by_c = {}

            def proj_thunks(c):
                cs = slice(SC * c, SC * (c + 1))
                qrot_by_c[c] = {}

                def q_chain(p):
                    def f():
                        t = qrot_pool.tile(
                            [128, SC], BF16, tag=f"qr{p}", name=f"qr{p}"
                        )
                        qrot_by_c[c][p] = t
                        q_ps = ring.tile([128, SC], F32, tag="mm", name="q_ps")
                        for dt in range(DT):
                            nc.tensor.matmul(
                                q_ps,
                                wq_sb[dt][:, 128 * p : 128 * (p + 1)],
                                xt_sb[dt][:, cs],
                                start=(dt == 0),
                                stop=(dt == DT - 1),
                            )
                        rope(q_ps, 128, cs, t)
                    return f

                def kv_chain():
                    kv_ps = ring.tile([128, SC], F32, tag="mm", name="kv_ps")
                    for dt in range(DT):
                        nc.tensor.matmul(
                            kv_ps,
                            wkv_sb[dt],
                            xt_sb[dt][:, cs],
                            start=(dt == 0),
                            stop=(dt == DT - 1),
                        )
                    rope(kv_ps, 64, cs, krot[c][0:64, :])
                    # duplicate k to the upper partition half for odd heads
                    nc.sync.dma_start(out=krot[c][64:128, :], in_=krot[c][0:64, :])
                    # v: evacuate for later transposes
                    vT = vt_pool.tile([64, SC], BF16, tag="vT", name=f"vT{c}")
                    nc.vector.tensor_copy(vT, kv_ps[64:128, :])
                    kv_hold[c] = vT

                def v_trans(j):
                    def f():
                        with tc.high_priority(offset=-1500):
                            vt_ps = ring.tile([128, HD], BF16, tag="mm", name="vt_ps")
                            nc.tensor.transpose(
                                vt_ps,
                                kv_hold[c][:, 128 * j : 128 * (j + 1)],
                                ident_sb[0:64, 0:64],
                            )
                            nc.vector.tensor_copy(v_sb[4 * c + j][:, 0:HD], vt_ps)
                    return f

                return [q_chain(0), q_chain(1), kv_chain] + [
                    v_trans(j) for j in range(4)
                ]

            kv_hold = {}
            fin_hold = {}
            pending_wo = None
            # prologue: chunk 0 projections run up front
            for th in proj_thunks(0):
                th()

            for c in range(NCH):
                # background work to interleave into this chunk's attention:
                # wo blocks of chunk c-1 and projections of chunk c+1
                wo_th = []
                if pending_wo is not None:
                    pw = pending_wo
                    wo_th = [
                        (lambda qb: lambda: emit_wo_qb(pw[0], pw[1], qb))(qb)
                        for qb in range(4)
                    ]
                pj_th = proj_thunks(c + 1) if c + 1 < NCH else []
                bg = [x for pair in zip(
                    wo_th + [None] * len(pj_th), pj_th + [None] * len(wo_th)
                ) for x in pair if x is not None]

                bg = fin_hold.pop("t", []) + bg
                qrot = qrot_by_c[c]
                # ---- attention ----
                outT_c = [
                    outT_pool.tile([128, SC], BF16, tag=f"oT{r}", name=f"oT{r}")
                    for r in range(2)
                ]
                if dbg and c == 0:
                    nc.sync.dma_start(out=dqr, in_=qrot[0])
                    nc.sync.dma_start(out=dkr, in_=krot[0])
                    nc.sync.dma_start(out=dv0, in_=v_sb[0])
                nkb = 4 * c + 4
                # one flattened, software-pipelined stream over (hp, kb):
                # PV(kb) trails scores by 2 kb groups so the PE never waits on
                # exp; retirement normalization is DVE-only and inline; all 8
                # transposes retire at chunk end (deps long satisfied by then)
                hp_state = {}
                o_ns = {}
                retire_q = []

                def emit_pv(hp, kb, j, pt2, hp_state=hp_state, retire_q=retire_q):
                    if kb == 0:
                        # qb3 slot starts at col 512 so no 65-col accumulation
                        # region crosses a PSUM bank boundary (start=True
                        # zeroing does not reach across banks)
                        pv_hp = pv.tile([128, 642], F32, tag="pv", name="pv_hp")
                        base = {0: 0, 1: 130, 2: 260, 3: 512}
                        hp_state[hp] = {
                            qb: pv_hp[:, base[qb] : base[qb] + 130]
                            for qb in range(4)
                        }
                    slots = hp_state[hp]
                    for qb in range(max(j, 0), 4):
                        for hi in range(2):
                            # PSUM start=True zeroes a whole 2KB zero-region
                            # (bank), so only ONE group may open per bank:
                            # start on the first matmul into each bank
                            # (qb0 -> bank0, qb3 -> bank1), stop on the last
                            # (qb2/qb3 diag, hi=1). Intermediate slots get
                            # first-touch-overwrite via per-byte pending-zero.
                            nc.tensor.matmul(
                                slots[qb][:, 65 * hi : 65 * (hi + 1)],
                                pt2[:, SC * hi + 128 * qb : SC * hi + 128 * (qb + 1)],
                                v_sb[kb][:, 0 : HD + 1],
                                start=(kb == 0 and hi == 0 and qb in (0, 3)),
                                stop=(j == qb and hi == 1 and qb in (2, 3)),
                                skip_group_check=True,
                            )
                        if j == qb:
                            retire_q.append((hp, qb))

                def run_retires(retire_q=retire_q, hp_state=hp_state, o_ns=o_ns, c=c):
                    while retire_q:
                        hp_, qb = retire_q.pop(0)
                        slot = hp_state[hp_][qb]
                        rs = slot[:, :].ap[0][0]
                        den = AP(
                            slot[:, :].tensor,
                            slot[0:1, 0:1].offset + HD,
                            [[rs, 128], [65, 2]],
                        )
                        rec = rec_pool.tile([128, 2], F32, tag="rec", name="rec")
                        with nc.allow_low_precision(reason="softmax denom"):
                            nc.vector.reciprocal(rec, den)
                        o_n = on_pool.tile([128, 128], BF16, tag="on", name="o_n")
                        for hi in range(2):
                            nc.vector.tensor_scalar(
                                o_n[:, 64 * hi : 64 * (hi + 1)],
                                slot[:, 65 * hi : 65 * hi + HD],
                                rec[:, hi : hi + 1],
                                None,
                                mul,
                            )
                        o_ns[(hp_, qb)] = o_n
                        if dbg and c == 0 and hp_ == 0 and qb == 0:
                            nc.sync.dma_start(out=don, in_=o_n)

                pends = []
                for hp in range(2):
                    for kb in range(nkb):
                        j = kb - 4 * c
                        col0 = max(0, 128 * j)
                        s2 = ring.tile([128, 2 * SC], F32, tag="mm", name="s2")
                        pt2 = pt_pool.tile([128, 2 * SC], BF16, tag="pt", name="pt2")
                        for hi in range(2):
                            nc.tensor.matmul(
                                s2[:, SC * hi + col0 : SC * (hi + 1)],
                                krot[kb // 4][
                                    64 * hi : 64 * (hi + 1),
                                    128 * (kb % 4) : 128 * (kb % 4 + 1),
                                ],
                                qrot[hp][64 * hi : 64 * (hi + 1), col0:SC],
                                start=True,
                                stop=True,
                            )
                        if col0 == 0:
                            nc.scalar.activation(pt2, s2, Exp, scale=0.125)
                        else:
                            s2v = s2.rearrange("a (t q) -> a t q", t=2)[:, :, col0:]
                            pt2v = pt2.rearrange("a (t q) -> a t q", t=2)[:, :, col0:]
                            nc.scalar.activation(pt2v, s2v, Exp, scale=0.125)
                        if j >= 0:
                            for hi in range(2):
                                w = slice(SC * hi + col0, SC * hi + col0 + 128)
                                nc.vector.tensor_tensor(
                                    pt2[:, w], pt2[:, w], m01_sb, mul
                                )
                        if dbg and c == 0 and hp == 0 and kb == 0:
                            nc.sync.dma_start(out=dpt, in_=pt2)
                        pends.append((hp, kb, j, pt2))
                        cap = 4
                        if len(pends) > cap:
                            emit_pv(*pends.pop(0))
                        if kb == 1 and retire_q:
                            run_retires()
                        # slot one background thunk (next-chunk proj / prev-chunk
                        # wo) between kb groups to fill exp-latency bubbles
                        # (small chunks have few kb slots, so pop every kb there)
                        if bg and (kb % 2 == 1 or c == 1):
                            bg.pop(0)()
                for p_ in pends:
                    emit_pv(*p_)
                if c + 1 < NCH:
                    def tr_bundle(oc, onsd, hp, cc):
                        def f():
                            for qb in range(4):
                                tr_ps = ring.tile(
                                    [128, 128], BF16, tag="mm", name="tr_ps"
                                )
                                nc.tensor.transpose(tr_ps, onsd[(hp, qb)], ident_sb)
                                nc.vector.tensor_copy(
                                    oc[hp][:, 128 * qb : 128 * (qb + 1)], tr_ps
                                )
                            if dbg and cc == 0 and hp == 0:
                                nc.sync.dma_start(out=doT, in_=oc[0])
                        return f

                    fin_hold["t"] = [run_retires] + [
                        tr_bundle(outT_c, o_ns, hp, c) for hp in range(2)
                    ]
                else:
                    # last chunk: drain leftover background first, then
                    # retire all, then per-qb transpose -> wo -> ship
                    while bg:
                        bg.pop(0)()
                    run_retires()
                    for qb in range(4):
                        for hp in range(2):
                            tr_ps = ring.tile([128, 128], BF16, tag="mm", name="tr_ps")
                            nc.tensor.transpose(tr_ps, o_ns[(hp, qb)], ident_sb)
                            nc.vector.tensor_copy(
                                outT_c[hp][:, 128 * qb : 128 * (qb + 1)], tr_ps
                            )
                        emit_wo_qb(c, outT_c, qb)
                # drain leftover background thunks (small chunks have few slots)
                for th in bg:
                    th()
                pending_wo = (c, outT_c)

    if split:
        split_excess_waits(nc)
    return nc


def split_excess_waits(nc, max_waits=1):
    """This container's walrus codegen supports one semaphore wait per
    instruction. Hoist excess waits onto NOPs injected just before, on the
    same engine (engine program order preserves the semantics)."""
    n_split = 0
    for fn in nc.m.functions:
        for bb in fn.blocks:
            insts = bb.instructions
            new = []
            for inst in insts:
                si = inst.sync_info
                waits = list(si.on_wait) if si is not None and si.on_wait else []
                if len(waits) > max_waits:
                    n_split += 1
                    extra, keep = waits[:-max_waits], waits[-max_waits:]
                    for k in range(0, len(extra), max_waits):
                        nop = mybir.InstNoOp(
                            name=nc.get_next_instruction_name(), ins=[], outs=[]
                        )
                        nop.engine = inst.engine
                        nop.sync_info = mybir.SyncInfo(
                            on_wait=extra[k : k + max_waits], on_update=[]
                        )
                        new.append(nop)
                    inst.sync_info = mybir.SyncInfo(
                        on_wait=keep,
                        on_update=list(si.on_update) if si.on_update else [],
                    )
                new.append(inst)
            bb.instructions = new
    return n_split


def _host_shards(x, wq, wk, wv, wo, freqs_cos, freqs_sin, mask):
    deint = np.concatenate([np.arange(0, HD, 2), np.arange(1, HD, 2)])

    cosb = np.tile(freqs_cos.T, (4, 1)).astype(BF)         # [128, S] rows r: cos_{r%32}
    sinb = np.tile(freqs_sin.T, (4, 1)).astype(BF)
    # signed swap permutation: (P @ t)[r] = sign(r) * t[r ^ 32],
    # sign = -1 for r%64 < 32 (the "or = tr*c - ti*s" half), +1 otherwise
    sign = np.where((np.arange(128) % 64) < 32, -1.0, 1.0).astype(np.float32)
    partner = np.arange(128) ^ 32
    P = np.zeros((128, 128), dtype=np.float32)
    P[partner, np.arange(128)] = sign
    m01 = (np.arange(128)[None, :] >= np.arange(128)[:, None]).astype(np.float32)
    ident = np.eye(128, dtype=np.float32)

    xts = [np.ascontiguousarray(x[b].T).astype(BF) for b in range(B)]

    in_maps = []
    for core in range(8):
        b, g = divmod(core, KVH)
        wq_g = wq[:, g * GH * HD : (g + 1) * GH * HD]
        wq_d = np.concatenate(
            [wq_g[:, h * HD + deint] for h in range(GH)], axis=1
        ).astype(BF)
        wk_g = wk[:, g * HD : (g + 1) * HD][:, deint]
        wv_g = wv[:, g * HD : (g + 1) * HD]
        wkv_g = np.concatenate([wk_g, wv_g], axis=1).astype(BF)
        wo_g = wo[g * GH * HD : (g + 1) * GH * HD, :].astype(BF)
        in_maps.append(
            {
                "xt": xts[b],
                "wq": np.ascontiguousarray(wq_d),
                "wkv": np.ascontiguousarray(wkv_g),
                "wo": np.ascontiguousarray(wo_g),
                "cos4": np.ascontiguousarray(cosb),
                "sin4": np.ascontiguousarray(sinb),
                "m01": m01.astype(BF),
                "pmat": P.astype(BF),
                "ident": ident.astype(BF),
            }
        )
    return in_maps


_NC_CACHE = None


def get_nc():
    global _NC_CACHE
    if _NC_CACHE is None:
        _NC_CACHE = build_nc()
    return _NC_CACHE


def kernel(x, wq, wk, wv, wo, freqs_cos, freqs_sin, mask):
    in_maps = _host_shards(
        np.asarray(x),
        np.asarray(wq),
        np.asarray(wk),
        np.asarray(wv),
        np.asarray(wo),
        np.asarray(freqs_cos),
        np.asarray(freqs_sin),
        np.asarray(mask),
    )
    nc = get_nc()
    res = run_bass_kernel_spmd(nc, in_maps, core_ids=list(range(8)))
    parts = [np.asarray(res.results[i]["out"], dtype=np.float32) for i in range(8)]
    out = np.stack(
        [
            parts[0] + parts[1] + parts[2] + parts[3],
            parts[4] + parts[5] + parts[6] + parts[7],
        ]
    ).astype(np.float32)
    return out


# revision 4
# speedup vs baseline: 1.0207x; 1.0026x over previous
"""GQA causal attention (B=2, S=2048, D=1024, H=16, KVH=4) on 8 trn2 cores.

Core = (b, g): batch b, kv-head group g. Each core projects q (4 heads,
column-parallel), k/v (1 kv head), applies RoPE, runs causal attention, and
computes a row-parallel wo partial (bf16); the host sums 4 partials per batch.

v2 design (vs baseline): everything bf16 on the PE/DMA path (PSUM accum f32):
- bf16 matmuls run 1 cycle/row at ANY free size (no fp32r N>=256 penalty), so
  diagonal score blocks narrow to their exact valid windows.
- RoPE partner comes from a single 128x128 signed-permutation matmul on the
  projected q (1 extra N=512 matmul) instead of re-projecting against swapped
  weight columns (8 matmuls) — PE cost for rope drops 8x.
- k and v share one projection chain ([wk_deint | wv] stationary, k on
  partitions 0:63, v on 64:127); k is duplicated to the upper half by one
  SBUF-to-SBUF DMA for the odd heads' score matmuls.
- PV runs transposed: out[q,65] per (head, q-block) with probs as stationary —
  full 128-partition contraction (vs M=65 half-idle) halves PV column count;
  the 65th v column of ones accumulates the softmax denominators.
- normalization via per-partition tensor_scalar (no DRAM broadcast
  round-trip), then a PE transpose builds outT for the row-parallel wo.
- elementwise work is spread across DVE and GPSIMD so ACT only runs the exps.
PSUM: ring pool (scores/proj/wo, 2x2 banks) + transpose pool (1 bank) +
PV accumulators (8 slots, 3 banks) = 8 banks exactly.
"""

import numpy as np
import ml_dtypes

import concourse.bass as bass
import concourse.mybir as mybir
import concourse.tile as tile
from concourse.ap import AP
from concourse.bass_utils import run_bass_kernel_spmd

B, S, D = 2, 2048, 1024
H, KVH, HD = 16, 4, 64
GH = H // KVH          # 4 q heads per core
SC = 512               # q-chunk
NCH = S // SC          # 4 chunks
DT = D // 128          # 8 contraction tiles
F32 = mybir.dt.float32
F32R = mybir.dt.float32r
BF16 = mybir.dt.bfloat16
BF = ml_dtypes.bfloat16
Exp = mybir.ActivationFunctionType.Exp
MUL = None  # set in build


def build_nc(split=True, dbg=False):
    nc = bass.Bass("TRN2", target_bir_lowering=False, debug=False, num_devices=1)
    mul = mybir.AluOpType.mult
    add = mybir.AluOpType.add

    xt = nc.dram_tensor("xt", [D, S], BF16, kind="ExternalInput").ap()
    wq = nc.dram_tensor("wq", [D, GH * HD], BF16, kind="ExternalInput").ap()
    wkv = nc.dram_tensor("wkv", [D, 2 * HD], BF16, kind="ExternalInput").ap()
    wk2 = nc.dram_tensor("wk2", [D, 2 * HD], BF16, kind="ExternalInput").ap()
    wo = nc.dram_tensor("wo", [GH * HD, D], BF16, kind="ExternalInput").ap()
    cos4 = nc.dram_tensor("cos4", [128, S], BF16, kind="ExternalInput").ap()
    sin4 = nc.dram_tensor("sin4", [128, S], BF16, kind="ExternalInput").ap()
    m01 = nc.dram_tensor("m01", [128, 128], BF16, kind="ExternalInput").ap()
    pmat = nc.dram_tensor("pmat", [128, 128], BF16, kind="ExternalInput").ap()
    ident = nc.dram_tensor("ident", [128, 128], BF16, kind="ExternalInput").ap()
    out = nc.dram_tensor("out", [S, D], BF16, kind="ExternalOutput").ap()
    if dbg:
        dqr = nc.dram_tensor("dqr", [128, SC], BF16, kind="ExternalOutput").ap()
        dkr = nc.dram_tensor("dkr", [128, SC], BF16, kind="ExternalOutput").ap()
        dv0 = nc.dram_tensor("dv0", [128, HD + 1], BF16, kind="ExternalOutput").ap()
        dpt = nc.dram_tensor("dpt", [128, 2 * SC], BF16, kind="ExternalOutput").ap()
        don = nc.dram_tensor("don", [128, 128], BF16, kind="ExternalOutput").ap()
        doT = nc.dram_tensor("doT", [128, SC], BF16, kind="ExternalOutput").ap()

    with tile.TileContext(nc) as tc:
        from contextlib import ExitStack

        with ExitStack() as ctx:
            singles = ctx.enter_context(tc.tile_pool(name="singles", bufs=1))
            persist = ctx.enter_context(tc.tile_pool(name="persist", bufs=1))
            qd_pool = ctx.enter_context(tc.tile_pool(name="qd", bufs=2))
            tc_pool = ctx.enter_context(tc.tile_pool(name="tcs", bufs=2))
            qrot_pool = ctx.enter_context(tc.tile_pool(name="qrot", bufs=2))
            vt_pool = ctx.enter_context(tc.tile_pool(name="vt", bufs=2))
            pt_pool = ctx.enter_context(tc.tile_pool(name="pt", bufs=8))
            rec_pool = ctx.enter_context(tc.tile_pool(name="rec", bufs=2))
            on_pool = ctx.enter_context(tc.tile_pool(name="on", bufs=8))
            outT_pool = ctx.enter_context(tc.tile_pool(name="outT", bufs=2))
            stage_pool = ctx.enter_context(tc.tile_pool(name="stage", bufs=2))
            ring = ctx.enter_context(tc.tile_pool(name="ring", bufs=3, space="PSUM"))
            pv = ctx.enter_context(tc.tile_pool(name="pv", bufs=1, space="PSUM"))

            # ---- batched loads: the 128-partition-row tiles of each DRAM
            # tensor are packed side by side in one SBUF tile and shipped with
            # a single 3-dim DMA (HWDGE issue slots, not bytes, dominate the
            # startup). Chunk-0's x/cos/sin columns come first so the first
            # projection chain starts within a few us. ----
            def load_packed(name, dram, rows, width, eng, cols=None):
                nblk = rows // 128
                t = singles.tile([128, nblk * width], BF16, tag=name, name=name)
                c0, c1 = (0, width) if cols is None else cols
                w = c1 - c0
                in_ap = AP(dram.tensor, c0, [[width, 128], [128 * width, nblk], [1, w]])
                rs = t[:, :].ap[0][0]
                out_ap = AP(t[:, :].tensor, t[0:1, 0:1].offset + c0,
                            [[rs, 128], [width, nblk], [1, w]])
                eng.dma_start(out=out_ap, in_=in_ap)
                return t

            wq_all = load_packed("wqa", wq, D, GH * HD, nc.sync)
            xt_all = load_packed("xta", xt, D, S, nc.scalar, cols=(0, SC))
            sincos_sb = singles.tile([128, 2 * S], BF16, tag="sincos")
            sin_sb = sincos_sb[:, 0:S]
            cos_sb = sincos_sb[:, S : 2 * S]
            nc.sync.dma_start(out=sin_sb[0:128, 0:SC], in_=sin4[:, 0:SC])
            nc.sync.dma_start(out=cos_sb[0:128, 0:SC], in_=cos4[:, 0:SC])
            pmat_sb = singles.tile([128, 128], BF16, tag="pmat")
            nc.sync.dma_start(out=pmat_sb, in_=pmat)
            wk2_all = load_packed("wk2a", wk2, D, 2 * HD, nc.sync)
            wkv_all = load_packed("wkva", wkv, D, 2 * HD, nc.sync)
            ident_sb = singles.tile([128, 128], BF16, tag="ident")
            nc.sync.dma_start(out=ident_sb, in_=ident)
            m01_sb = singles.tile([128, 128], BF16, tag="m01")
            nc.sync.dma_start(out=m01_sb, in_=m01)
            # bulk loads are deprioritized so the scheduler keeps them off
            # the startup critical path (DMA_ENGINES is a serial resource);
            # the x remainder ships in per-chunk pieces to bound head-of-line
            # blocking
            with tc.high_priority(offset=-100000):
                nc.sync.dma_start(out=cos_sb[0:128, SC:S], in_=cos4[:, SC:S])
                nc.sync.dma_start(out=sin_sb[0:128, SC:S], in_=sin4[:, SC:S])
                rs = xt_all[:, :].ap[0][0]
                # half-chunk pieces bound DMA_ENGINES head-of-line blocking of
                # small critical transfers (e.g. the krot dup) to ~0.7us
                for piece in range(2, 2 * NCH):
                    c0 = (SC // 2) * piece
                    in2 = AP(xt.tensor, c0, [[S, 128], [128 * S, DT], [1, SC // 2]])
                    out2 = AP(xt_all[:, :].tensor, xt_all[0:1, 0:1].offset + c0,
                              [[rs, 128], [S, DT], [1, SC // 2]])
                    nc.scalar.dma_start(out=out2, in_=in2)
                wo_all = load_packed("woa", wo, GH * HD, D, nc.sync)

            junk_sb = singles.tile([128, 64], BF16, tag="junk")
            nc.vector.memset(junk_sb, 0.0)
            for w in range(40):
                junk_ps = pv.tile([128, 642], F32, tag="pv", name="junk_ps")
                nc.tensor.matmul(
                    junk_ps[0:64, 0:64], junk_sb, junk_sb, start=True, stop=True
                )

            wq_sb = [wq_all[:, GH * HD * dt : GH * HD * (dt + 1)] for dt in range(DT)]
            xt_sb = [xt_all[:, S * dt : S * (dt + 1)] for dt in range(DT)]
            wkv_sb = [wkv_all[:, 2 * HD * dt : 2 * HD * (dt + 1)] for dt in range(DT)]
            wk2_sb = [wk2_all[:, 2 * HD * dt : 2 * HD * (dt + 1)] for dt in range(DT)]
            wo_sb = [wo_all[:, D * r : D * (r + 1)] for r in range(2)]

            # ---- persistent activations ----
            krot = [
                persist.tile([128, SC], BF16, tag=f"krot{c}", name=f"krot{c}")
                for c in range(NCH)
            ]
            v_sb = [
                persist.tile([128, HD + 1], BF16, tag=f"v{kb}", name=f"v{kb}")
                for kb in range(S // 128)
            ]
            for kb in range(S // 128):
                nc.gpsimd.memset(v_sb[kb][:, HD : HD + 1], 1.0)

            def rope(ps, rows, cs, dst):
                """dst = (ps*cos) + P @ (ps*sin), exploiting that cos/sin are
                partner-symmetric across the 32-row swap. All bf16 out."""
                d_sin = qd_pool.tile([rows, SC], BF16, tag="qd", name="d_sin")
                nc.vector.tensor_tensor(d_sin, ps[0:rows, :], sin_sb[0:rows, cs], mul)
                d_cos = tc_pool.tile([rows, SC], BF16, tag="tc", name="d_cos")
                nc.vector.tensor_tensor(d_cos, ps[0:rows, :], cos_sb[0:rows, cs], mul)
                s_ps = ring.tile([rows, SC], F32, tag="mm", name="s_ps")
                nc.tensor.matmul(
                    s_ps, pmat_sb[0:rows, 0:rows], d_sin, start=True, stop=True
                )
                nc.vector.tensor_tensor(dst, s_ps, d_cos, add)

            stage_hold = {}

            def emit_wo_qb(c, outT_c, qb):
                """One 128-row output block: 2 accumulating matmuls into a pv-pool
                psum tile, evacuated to a shared bf16 stage (DVE/ACT alternating);
                one batched DMA per chunk ships all 4 blocks."""
                w_ps = ring.tile([128, D], F32, tag="mm", name="w_ps")
                for n in range(2):
                    for r in range(2):
                        nc.tensor.matmul(
                            w_ps[:, 512 * n : 512 * (n + 1)],
                            outT_c[r][:, 128 * qb : 128 * (qb + 1)],
                            wo_sb[r][:, 512 * n : 512 * (n + 1)],
                            start=(r == 0),
                            stop=(r == 1),
                        )
                if qb == 0:
                    stage_hold["t"] = stage_pool.tile(
                        [128, 4 * D], BF16, tag="stage", name="stage"
                    )
                stage = stage_hold["t"]
                if c + 1 < NCH:
                    # low-priority on DVE: the scheduler slots these 1.2us
                    # copies into DVE idle windows off the ACT exp stream
                    with tc.high_priority(offset=-2000):
                        nc.vector.tensor_copy(stage[:, D * qb : D * (qb + 1)], w_ps)
                else:
                    # critical tail: split halves across DVE and ACT
                    h = D // 2
                    nc.vector.tensor_copy(
                        stage[:, D * qb : D * qb + h], w_ps[:, 0:h]
                    )
                    nc.scalar.copy(stage[:, D * qb + h : D * (qb + 1)], w_ps[:, h:D])
                r0 = SC * c + 128 * qb
                if c + 1 < NCH:
                    eng = nc.sync if qb % 2 == 0 else nc.gpsimd
                else:
                    eng = nc.sync if qb % 2 == 0 else nc.scalar
                eng.dma_start(out=out[r0 : r0 + 128, :], in_=stage[:, D * qb : D * (qb + 1)])

            # ---- deferred-emission machinery: projection chains for chunk c+1
            # and wo blocks for chunk c-1 are emitted as background thunks
            # interleaved into chunk c's kb loop, so the PE fills exp-latency
            # bubbles with projection/wo work and ACT never starves ----
            qrot_by_c = {}

            def proj_thunks(c):
                cs = slice(SC * c, SC * (c + 1))
                qrot_by_c[c] = {}

                def q_chain(p):
                    def f():
                        t = qrot_pool.tile(
                            [128, SC], BF16, tag=f"qr{p}", name=f"qr{p}"
                        )
                        qrot_by_c[c][p] = t
                        q_ps = ring.tile([128, SC], F32, tag="mm", name="q_ps")
                        for dt in range(DT):
                            nc.tensor.matmul(
                                q_ps,
                                wq_sb[dt][:, 128 * p : 128 * (p + 1)],
                                xt_sb[dt][:, cs],
                                start=(dt == 0),
                                stop=(dt == DT - 1),
                            )
                        rope(q_ps, 128, cs, t)
                    return f

                def kv_chain():
                    if c == 0:
                        # chunk 0: k duplicated in the stationary (M-dup is
                        # free on the PE) so no dup DMA sits on the startup
                        # critical path; v comes from its own chain, which
                        # overlaps the DMA-gated warmup
                        k2_ps = ring.tile([128, SC], F32, tag="mm", name="k2_ps")
                        for dt in range(DT):
                            nc.tensor.matmul(
                                k2_ps,
                                wk2_sb[dt],
                                xt_sb[dt][:, cs],
                                start=(dt == 0),
                                stop=(dt == DT - 1),
                            )
                        rope(k2_ps, 128, cs, krot[c])
                        v_ps = ring.tile([64, SC], F32, tag="mm", name="v_ps")
                        for dt in range(DT):
                            nc.tensor.matmul(
                                v_ps,
                                wkv_sb[dt][:, HD : 2 * HD],
                                xt_sb[dt][:, cs],
                                start=(dt == 0),
                                stop=(dt == DT - 1),
                            )
                        vT = vt_pool.tile([64, SC], BF16, tag="vT", name=f"vT{c}")
                        nc.vector.tensor_copy(vT, v_ps)
                        kv_hold[c] = vT
                        return
                    kv_ps = ring.tile([128, SC], F32, tag="mm", name="kv_ps")
                    for dt in range(DT):
                        nc.tensor.matmul(
                            kv_ps,
                            wkv_sb[dt],
                            xt_sb[dt][:, cs],
                            start=(dt == 0),
                            stop=(dt == DT - 1),
                        )
                    rope(kv_ps, 64, cs, krot[c][0:64, :])
                    # duplicate k to the upper partition half for odd heads
                    with tc.high_priority(offset=-1500):
                        nc.sync.dma_start(out=krot[c][64:128, :], in_=krot[c][0:64, :])
                    # v: evacuate for later transposes
                    vT = vt_pool.tile([64, SC], BF16, tag="vT", name=f"vT{c}")
                    nc.vector.tensor_copy(vT, kv_ps[64:128, :])
                    kv_hold[c] = vT

                def v_trans(j):
                    def f():
                        with tc.high_priority(offset=-1500):
                            vt_ps = ring.tile([128, HD], BF16, tag="mm", name="vt_ps")
                            nc.tensor.transpose(
                                vt_ps,
                                kv_hold[c][:, 128 * j : 128 * (j + 1)],
                                ident_sb[0:64, 0:64],
                            )
                            nc.vector.tensor_copy(v_sb[4 * c + j][:, 0:HD], vt_ps)
                    return f

                return [q_chain(0), q_chain(1), kv_chain] + [
                    v_trans(j) for j in range(4)
                ]

            kv_hold = {}
            fin_hold = {}
            pending_wo = None
            # prologue: chunk 0 projections run up front
            for th in proj_thunks(0):
                th()

            for c in range(NCH):
                # background work to interleave into this chunk's attention:
                # wo blocks of chunk c-1 and projections of chunk c+1
                wo_th = []
                if pending_wo is not None:
                    pw = pending_wo
                    wo_th = [
                        (lambda qb: lambda: emit_wo_qb(pw[0], pw[1], qb))(qb)
                        for qb in range(4)
                    ]
                pj_th = proj_thunks(c + 1) if c + 1 < NCH else []
                bg = [x for pair in zip(
                    wo_th + [None] * len(pj_th), pj_th + [None] * len(wo_th)
                ) for x in pair if x is not None]

                bg = fin_hold.pop("t", []) + bg
                qrot = qrot_by_c[c]
                # ---- attention ----
                outT_c = [
                    outT_pool.tile([128, SC], BF16, tag=f"oT{r}", name=f"oT{r}")
                    for r in range(2)
                ]
                if dbg and c == 0:
                    nc.sync.dma_start(out=dqr, in_=qrot[0])
                    nc.sync.dma_start(out=dkr, in_=krot[0])
                    nc.sync.dma_start(out=dv0, in_=v_sb[0])
                nkb = 4 * c + 4
                # one flattened, software-pipelined stream over (hp, kb):
                # PV(kb) trails scores by 2 kb groups so the PE never waits on
                # exp; retirement normalization is DVE-only and inline; all 8
                # transposes retire at chunk end (deps long satisfied by then)
                hp_state = {}
                o_ns = {}
                retire_q = []

                def emit_pv(hp, kb, j, pt2, hp_state=hp_state, retire_q=retire_q):
                    if kb == 0:
                        # qb3 slot starts at col 512 so no 65-col accumulation
                        # region crosses a PSUM bank boundary (start=True
                        # zeroing does not reach across banks)
                        pv_hp = pv.tile([128, 642], F32, tag="pv", name="pv_hp")
                        base = {0: 0, 1: 130, 2: 260, 3: 512}
                        hp_state[hp] = {
                            qb: pv_hp[:, base[qb] : base[qb] + 130]
                            for qb in range(4)
                        }
                    slots = hp_state[hp]
                    for qb in range(max(j, 0), 4):
                        for hi in range(2):
                            # PSUM start=True zeroes a whole 2KB zero-region
                            # (bank), so only ONE group may open per bank:
                            # start on the first matmul into each bank
                            # (qb0 -> bank0, qb3 -> bank1), stop on the last
                            # (qb2/qb3 diag, hi=1). Intermediate slots get
                            # first-touch-overwrite via per-byte pending-zero.
                            nc.tensor.matmul(
                                slots[qb][:, 65 * hi : 65 * (hi + 1)],
                                pt2[:, SC * hi + 128 * qb : SC * hi + 128 * (qb + 1)],
                                v_sb[kb][:, 0 : HD + 1],
                                start=(kb == 0 and hi == 0 and qb in (0, 3)),
                                stop=(j == qb and hi == 1 and qb in (2, 3)),
                                skip_group_check=True,
                            )
                        if j == qb:
                            retire_q.append((hp, qb))

                def run_retires(retire_q=retire_q, hp_state=hp_state, o_ns=o_ns, c=c):
                    while retire_q:
                        hp_, qb = retire_q.pop(0)
                        slot = hp_state[hp_][qb]
                        rs = slot[:, :].ap[0][0]
                        den = AP(
                            slot[:, :].tensor,
                            slot[0:1, 0:1].offset + HD,
                            [[rs, 128], [65, 2]],
                        )
                        rec = rec_pool.tile([128, 2], F32, tag="rec", name="rec")
                        with nc.allow_low_precision(reason="softmax denom"):
                            nc.vector.reciprocal(rec, den)
                        o_n = on_pool.tile([128, 128], BF16, tag="on", name="o_n")
                        for hi in range(2):
                            nc.vector.tensor_scalar(
                                o_n[:, 64 * hi : 64 * (hi + 1)],
                                slot[:, 65 * hi : 65 * hi + HD],
                                rec[:, hi : hi + 1],
                                None,
                                mul,
                            )
                        o_ns[(hp_, qb)] = o_n
                        if dbg and c == 0 and hp_ == 0 and qb == 0:
                            nc.sync.dma_start(out=don, in_=o_n)

                pends = []
                for hp in range(2):
                    for kb in range(nkb):
                        j = kb - 4 * c
                        col0 = max(0, 128 * j)
                        s2 = ring.tile([128, 2 * SC], F32, tag="mm", name="s2")
                        pt2 = pt_pool.tile([128, 2 * SC], BF16, tag="pt", name="pt2")
                        for hi in range(2):
                            nc.tensor.matmul(
                                s2[:, SC * hi + col0 : SC * (hi + 1)],
                                krot[kb // 4][
                                    64 * hi : 64 * (hi + 1),
                                    128 * (kb % 4) : 128 * (kb % 4 + 1),
                                ],
                                qrot[hp][64 * hi : 64 * (hi + 1), col0:SC],
                                start=True,
                                stop=True,
                            )
                        if col0 == 0:
                            nc.scalar.activation(pt2, s2, Exp, scale=0.125)
                        else:
                            s2v = s2.rearrange("a (t q) -> a t q", t=2)[:, :, col0:]
                            pt2v = pt2.rearrange("a (t q) -> a t q", t=2)[:, :, col0:]
                            nc.scalar.activation(pt2v, s2v, Exp, scale=0.125)
                        if j >= 0:
                            for hi in range(2):
                                w = slice(SC * hi + col0, SC * hi + col0 + 128)
                                nc.vector.tensor_tensor(
                                    pt2[:, w], pt2[:, w], m01_sb, mul
                                )
                        if dbg and c == 0 and hp == 0 and kb == 0:
                            nc.sync.dma_start(out=dpt, in_=pt2)
                        pends.append((hp, kb, j, pt2))
                        cap = 4
                        if len(pends) > cap:
                            emit_pv(*pends.pop(0))
                        if kb == 1 and retire_q:
                            run_retires()
                        # slot one background thunk (next-chunk proj / prev-chunk
                        # wo) between kb groups to fill exp-latency bubbles
                        # (small chunks have few kb slots, so pop every kb there)
                        if bg and (kb % 2 == 1 or c == 1):
                            bg.pop(0)()
                for p_ in pends:
                    emit_pv(*p_)
                if c + 1 < NCH:
                    def tr_bundle(oc, onsd, hp, cc):
                        def f():
                            for qb in range(4):
                                tr_ps = ring.tile(
                                    [128, 128], BF16, tag="mm", name="tr_ps"
                                )
                                nc.tensor.transpose(tr_ps, onsd[(hp, qb)], ident_sb)
                                nc.vector.tensor_copy(
                                    oc[hp][:, 128 * qb : 128 * (qb + 1)], tr_ps
                                )
                            if dbg and cc == 0 and hp == 0:
                                nc.sync.dma_start(out=doT, in_=oc[0])
                        return f

                    fin_hold["t"] = [run_retires] + [
                        tr_bundle(outT_c, o_ns, hp, c) for hp in range(2)
                    ]
                else:
                    # last chunk: drain leftover background first, then
                    # retire all, then per-qb transpose -> wo -> ship
                    while bg:
                        bg.pop(0)()
                    run_retires()
                    for qb in range(4):
                        for hp in range(2):
                            tr_ps = ring.tile([128, 128], BF16, tag="mm", name="tr_ps")
                            nc.tensor.transpose(tr_ps, o_ns[(hp, qb)], ident_sb)
                            nc.vector.tensor_copy(
                                outT_c[hp][:, 128 * qb : 128 * (qb + 1)], tr_ps
                            )
                        emit_wo_qb(c, outT_c, qb)
                # drain leftover background thunks (small chunks have few slots)
                for th in bg:
                    th()
                pending_wo = (c, outT_c)

    if split:
        split_excess_waits(nc)
    return nc


def split_excess_waits(nc, max_waits=1):
    """This container's walrus codegen supports one semaphore wait per
    instruction. Hoist excess waits onto NOPs injected just before, on the
    same engine (engine program order preserves the semantics)."""
    n_split = 0
    for fn in nc.m.functions:
        for bb in fn.blocks:
            insts = bb.instructions
            new = []
            for inst in insts:
                si = inst.sync_info
                waits = list(si.on_wait) if si is not None and si.on_wait else []
                if len(waits) > max_waits:
                    n_split += 1
                    extra, keep = waits[:-max_waits], waits[-max_waits:]
                    for k in range(0, len(extra), max_waits):
                        nop = mybir.InstNoOp(
                            name=nc.get_next_instruction_name(), ins=[], outs=[]
                        )
                        nop.engine = inst.engine
                        nop.sync_info = mybir.SyncInfo(
                            on_wait=extra[k : k + max_waits], on_update=[]
                        )
                        new.append(nop)
                    inst.sync_info = mybir.SyncInfo(
                        on_wait=keep,
                        on_update=list(si.on_update) if si.on_update else [],
                    )
                new.append(inst)
            bb.instructions = new
    return n_split


def _host_shards(x, wq, wk, wv, wo, freqs_cos, freqs_sin, mask):
    deint = np.concatenate([np.arange(0, HD, 2), np.arange(1, HD, 2)])

    cosb = np.tile(freqs_cos.T, (4, 1)).astype(BF)         # [128, S] rows r: cos_{r%32}
    sinb = np.tile(freqs_sin.T, (4, 1)).astype(BF)
    # signed swap permutation: (P @ t)[r] = sign(r) * t[r ^ 32],
    # sign = -1 for r%64 < 32 (the "or = tr*c - ti*s" half), +1 otherwise
    sign = np.where((np.arange(128) % 64) < 32, -1.0, 1.0).astype(np.float32)
    partner = np.arange(128) ^ 32
    P = np.zeros((128, 128), dtype=np.float32)
    P[partner, np.arange(128)] = sign
    m01 = (np.arange(128)[None, :] >= np.arange(128)[:, None]).astype(np.float32)
    ident = np.eye(128, dtype=np.float32)

    xts = [np.ascontiguousarray(x[b].T).astype(BF) for b in range(B)]

    in_maps = []
    for core in range(8):
        b, g = divmod(core, KVH)
        wq_g = wq[:, g * GH * HD : (g + 1) * GH * HD]
        wq_d = np.concatenate(
            [wq_g[:, h * HD + deint] for h in range(GH)], axis=1
        ).astype(BF)
        wk_g = wk[:, g * HD : (g + 1) * HD][:, deint]
        wv_g = wv[:, g * HD : (g + 1) * HD]
        wkv_g = np.concatenate([wk_g, wv_g], axis=1).astype(BF)
        wk2_g = np.concatenate([wk_g, wk_g], axis=1).astype(BF)
        wo_g = wo[g * GH * HD : (g + 1) * GH * HD, :].astype(BF)
        in_maps.append(
            {
                "xt": xts[b],
                "wq": np.ascontiguousarray(wq_d),
                "wkv": np.ascontiguousarray(wkv_g),
                "wk2": np.ascontiguousarray(wk2_g),
                "wo": np.ascontiguousarray(wo_g),
                "cos4": np.ascontiguousarray(cosb),
                "sin4": np.ascontiguousarray(sinb),
                "m01": m01.astype(BF),
                "pmat": P.astype(BF),
                "ident": ident.astype(BF),
            }
        )
    return in_maps


_NC_CACHE = None


def get_nc():
    global _NC_CACHE
    if _NC_CACHE is None:
        _NC_CACHE = build_nc()
    return _NC_CACHE


def kernel(x, wq, wk, wv, wo, freqs_cos, freqs_sin, mask):
    in_maps = _host_shards(
        np.asarray(x),
        np.asarray(wq),
        np.asarray(wk),
        np.asarray(wv),
        np.asarray(wo),
        np.asarray(freqs_cos),
        np.asarray(freqs_sin),
        np.asarray(mask),
    )
    nc = get_nc()
    res = run_bass_kernel_spmd(nc, in_maps, core_ids=list(range(8)))
    parts = [np.asarray(res.results[i]["out"], dtype=np.float32) for i in range(8)]
    out = np.stack(
        [
            parts[0] + parts[1] + parts[2] + parts[3],
            parts[4] + parts[5] + parts[6] + parts[7],
        ]
    ).astype(np.float32)
    return out
